# revision 1
# baseline (speedup 1.0000x reference)
"""Trainium2 Bass kernel for nn_InpaintContextAttentionUnit.

Per-sample computation (B=8 samples -> 1 per NeuronCore):
  fm [512,512,16] -> avgpool(64x2) -> pooled [8,256,16]
  -> two masked 3x3 convs (middle row / middle col of kernel zeroed) + bias + relu
  -> bilinear upsample back to [512,512,16] (separable; half-pixel centers, edge clamp)
  -> out [512,512,48] = concat(fm, fm - row_up, fm - col_up)

Design:
  - pooling: PE matmul with a [128,2] block-mean matrix (H-reduce); W-pair add
    folded into a 2-matmul PSUM accumulation (even/odd x, strided rhs)
  - conv: per (branch, n-pair chunk): zero-init matmul + ~6 accumulating
    [16c,16f]x[16c,<=512] matmuls in PSUM; relu+bias on ACT; taps read from a
    wp-halo'd [16c, 8n x 258wp] buffer assembled via a DRAM bounce
  - W-upsample (x2, weights .25/.75): 2 strided scalar_tensor_tensor ops over an
    edge-replicated halo buffer
  - H-upsample (x64): PE matmul rw[8n, x] with host-built HUp interp matrix
    (row branch at partitions 0-7, col branch at 32-39 per base-partition rules)
  - combine: DVE subtract (fm - psum, strided APs) + ACT copy into interleaved
    [y, x, 48ch] staging tiles, contiguous 3 MiB DMAs out
  - the pooled->conv->upsample chain runs in bf16 (PE bf16 is ~4x faster than the
    fp32-emulation path); PSUM accumulation, fm passthrough, subtract, and the
    output stay fp32
All constant matrices are precomputed on host and passed as extra inputs.
"""

import numpy as np
import ml_dtypes

H, W, C, F = 512, 512, 16, 16
NPOOL = 8
WP = W // 2  # 256
CH_OUT = 3 * C  # 48

_cache = {}


def _host_consts(kernel, bias):
    """Build host-side constant matrices (bf16 for the PE-side constants)."""
    bf = ml_dtypes.bfloat16
    # pooling weights: [128, 2], 1/128 (exact in bf16) where row block matches
    poolw = np.zeros((128, 2), np.float32)
    poolw[:64, 0] = 1.0 / 128.0
    poolw[64:, 1] = 1.0 / 128.0
    # H-upsample matrix: hup[n, y] = weight of pooled row n for output row y
    # (k/64 weights are exact in bf16)
    hup = np.zeros((NPOOL, H), np.float32)
    scale = H // NPOOL
    for y in range(H):
        yf = (y + 0.5) / scale - 0.5
        i0 = int(np.floor(yf))
        w = yf - i0
        hup[min(max(i0, 0), NPOOL - 1), y] += 1.0 - w
        hup[min(max(i0 + 1, 0), NPOOL - 1), y] += w
    hup2 = np.zeros((40, H), np.float32)
    hup2[0:8] = hup
    hup2[32:40] = hup  # col-branch copy at base partition 32
    # conv taps: branch 0 (row conv): K[dn+1, dwp+1]; branch 1 (col): K[dwp+1, dn+1]
    taps0 = [(dn, dwp) for dn in (-1, 1) for dwp in (-1, 0, 1)]
    taps1 = [(dn, dwp) for dwp in (-1, 1) for dn in (-1, 0, 1)]
    kt = np.zeros((16, 13 * 16), np.float32)  # [c, tap*16+f]; slot 12 = zeros
    for i, (dn, dwp) in enumerate(taps0):
        kt[:, i * 16:(i + 1) * 16] = kernel[dn + 1, dwp + 1]
    for i, (dn, dwp) in enumerate(taps1):
        kt[:, (6 + i) * 16:(7 + i) * 16] = kernel[dwp + 1, dn + 1]
    bias2 = np.ascontiguousarray(bias.reshape(16, 1)).astype(np.float32)
    return (poolw.astype(bf), hup2.astype(bf), kt.astype(bf), bias2, taps0, taps1)


def _build_program(compile=True):
    import concourse.bass as bass
    import concourse.bacc as bacc
    import concourse.mybir as mybir
    import concourse.tile as tile

    dt = mybir.dt.float32
    db = mybir.dt.bfloat16
    nc = bacc.Bacc()

    fm_d = nc.declare_dram_parameter("feature_map", [H, W, C], dt, isOutput=False)
    poolw_d = nc.declare_dram_parameter("poolw", [128, 2], db, isOutput=False)
    hup_d = nc.declare_dram_parameter("hup", [40, H], db, isOutput=False)
    ktaps_d = nc.declare_dram_parameter("ktaps", [16, 208], db, isOutput=False)
    bias_d = nc.declare_dram_parameter("bias2", [16, 1], dt, isOutput=False)
    out_d = nc.declare_dram_parameter("out", [H, W, CH_OUT], dt, isOutput=True)

    taps0 = [(dn, dwp) for dn in (-1, 1) for dwp in (-1, 0, 1)]
    taps1 = [(dn, dwp) for dwp in (-1, 1) for dn in (-1, 0, 1)]
    taps_by_branch = [taps0, taps1]

    with tile.TileContext(nc) as tc:
        with (
            tc.tile_pool(name="consts", bufs=1) as cpool,
            tc.tile_pool(name="fm", bufs=2) as fmpool,
            tc.tile_pool(name="persist", bufs=1) as ppool,
        ):
            # ---- load constants ----
            poolw_t = cpool.tile([128, 2], db)
            nc.sync.dma_start(out=poolw_t[:], in_=poolw_d[:])
            hup_t = cpool.tile([40, H], db)
            nc.sync.dma_start(out=hup_t[:], in_=hup_d[:])
            ktaps_t = cpool.tile([16, 208], db)
            nc.sync.dma_start(out=ktaps_t[:], in_=ktaps_d[:])
            bias_t = cpool.tile([16, 1], dt)
            nc.sync.dma_start(out=bias_t[:], in_=bias_d[:])

            # rw [40, (16 f, 512 x)] bf16: partitions 0-7 row-branch, 32-39 col-branch
            rw_t = ppool.tile([40, 16 * 512], db)

            # ================= PASS A: pooling + conv + W-upsample =================
            with (
                tc.tile_pool(name="passA", bufs=1) as apool,
                tc.tile_pool(name="dram", bufs=1, space="DRAM") as dpool,
            ):
                # pooled_T [16 c, (8 n, 258 wp)] bf16, zero wp-halo; n-direction
                # zero-padding handled by clipped matmul n-ranges
                tpad_t = apool.tile([16, NPOOL * 258], db)
                tpad3 = tpad_t[:].rearrange("p (n w) -> p n w", w=258)

                # pooled_ncw [8 n, (16 c, 256 wp)] bf16, c-major
                ncw_t = apool.tile([NPOOL, 16 * WP], db)

                with tc.tile_pool(name="psA", bufs=1, space="PSUM") as psA:
                    for t in range(4):
                        # bf16 copy of fm for pooling only (SWDGE cast-DMA)
                        fmb_t = apool.tile([128, W * C], db, tag="fmA", bufs=4)
                        fmb3 = fmb_t[:].rearrange("p (x c) -> p x c", c=C)
                        nc.gpsimd.dma_start(out=fmb3, in_=fm_d[128 * t:128 * (t + 1)])

                        # stage [2, (c, wp)] bf16 on partitions 0-1
                        stage_t = apool.tile([2, 16 * WP], db, tag="stage", bufs=2)
                        stage3 = stage_t[:].rearrange("p (c w) -> p w c", c=16)
                        # fm viewed (xp, parity, c): W-pair add in PE accumulation;
                        # one whole-PSUM [2, 4096] tile per fm tile, each j-block
                        # lands in its own bank (512 f32 = 1 bank)
                        fmr = fmb_t[:].rearrange("p (xp two c) -> p xp two c", two=2, c=16)
                        ps = psA.tile([2, 8 * 512], dt, tag="pool")
                        for j in range(8):  # 32-xp chunks -> N=512
                            for par in range(2):
                                nc.tensor.matmul(
                                    ps[:, 512 * j:512 * (j + 1)], poolw_t[:],
                                    fmr[:, 32 * j:32 * (j + 1), par, :],
                                    start=(par == 0), stop=(par == 1),
                                )
                        ps3 = ps[:].rearrange("p (xp c) -> p xp c", c=16)
                        nc.vector.tensor_copy(stage3, ps3)
                        nc.sync.dma_start(out=ncw_t[2 * t:2 * t + 2, :], in_=stage_t[:])

                # pooled_ncw -> pooled_T (c to partitions) via DRAM bounce, adding
                # zero wp-halo columns (zeros sourced from hup rows 8-15, zero by
                # construction)
                ncw_dram = dpool.tile([NPOOL, 16 * 258], db)
                nd3 = ncw_dram[:].rearrange("n (c w) -> n c w", w=258)
                ncw3s = ncw_t[:].rearrange("p (c w) -> p c w", w=WP)
                nc.sync.dma_start(out=nd3[:, :, 1:257], in_=ncw3s)
                zsrc = hup_d[8:16, 0:16]  # [8, 16] zeros
                nc.sync.dma_start(out=nd3[:, :, 0:1], in_=zsrc)
                nc.sync.dma_start(out=nd3[:, :, 257:258], in_=zsrc)
                ncwd3 = ncw_dram[:].rearrange("n (c w) -> c n w", w=258)
                nc.sync.dma_start(out=tpad3, in_=ncwd3)

                # ---- conv branches ----
                conv_t = apool.tile([16, 2 * NPOOL * WP], db, tag="conv_t")
                psC_cm = tc.tile_pool(name="psConv", bufs=4, space="PSUM")
                psC_pool = psC_cm.__enter__()
                for b in range(2):
                    for ch in range(4):  # n-pair chunks: n in {2ch, 2ch+1}
                        n0 = 2 * ch
                        ps = psC_pool.tile([16, 2 * WP], dt, tag="conv")
                        # zero-init whole chunk (ktaps slot 12 = zeros)
                        nc.tensor.matmul(
                            ps[:], ktaps_t[:, 192:208], tpad3[:, n0:n0 + 2, 1:257],
                            start=True, stop=False, skip_group_check=True,
                        )
                        pieces = []
                        for i, (dn, dwp) in enumerate(taps_by_branch[b]):
                            nlo = max(n0, -dn)
                            nhi = min(n0 + 2, NPOOL - dn)
                            if nhi <= nlo:
                                continue
                            pieces.append((b * 6 + i, dn, dwp, nlo, nhi))
                        for k, (sl, dn, dwp, nlo, nhi) in enumerate(pieces):
                            nc.tensor.matmul(
                                ps[:, (nlo - n0) * WP:(nhi - n0) * WP],
                                ktaps_t[:, sl * 16:(sl + 1) * 16],
                                tpad3[:, nlo + dn:nhi + dn, 1 + dwp:257 + dwp],
                                start=False, stop=(k == len(pieces) - 1),
                                skip_group_check=True,
                            )
                        nc.scalar.activation(
                            out=conv_t[:, (b * NPOOL + n0) * WP:(b * NPOOL + n0 + 2) * WP],
                            in_=ps[:],
                            func=mybir.ActivationFunctionType.Relu,
                            bias=bias_t[:, 0:1],
                        )

                psC_cm.__exit__(None, None, None)
                # conv [16 f, (b, n, wp)] -> rop_pad [(b,n) parts, (16 f, 258 wp)]
                # via DRAM bounce (keeps consumer sync fan-in small)
                rop_t = apool.tile([40, 16 * 258], db)
                rop3 = rop_t[:].rearrange("p (f w) -> p f w", w=258)
                conv_dram = dpool.tile([16, 2 * NPOOL * WP], db)
                nc.sync.dma_start(out=conv_dram[:], in_=conv_t[:])
                cd4 = conv_dram[:].rearrange("f (b n w) -> b n f w", b=2, n=NPOOL)
                for b in range(2):
                    pg = 32 * b  # partition base: row->0, col->32
                    nc.sync.dma_start(out=rop3[pg:pg + 8, :, 1:257], in_=cd4[b])
                # edge replicate (W clamp)
                for pg in (0, 32):
                    nc.vector.tensor_copy(rop3[pg:pg + 8, :, 0:1], rop3[pg:pg + 8, :, 1:2])
                    nc.vector.tensor_copy(rop3[pg:pg + 8, :, 257:258], rop3[pg:pg + 8, :, 256:257])

                # W-upsample: rw[., f, 2k]   = 0.25*pad[k]   + 0.75*pad[k+1]
                #             rw[., f, 2k+1] = 0.25*pad[k+2] + 0.75*pad[k+1]
                t75_t = apool.tile([40, 16 * 258], db, tag="conv_t")
                t753 = t75_t[:].rearrange("p (f w) -> p f w", w=258)
                rw4 = rw_t[:].rearrange("p (f x two) -> p f x two", two=2, x=WP)
                for pg, eng in ((0, nc.vector), (32, nc.vector)):
                    eng.tensor_scalar_mul(
                        t75_t[pg:pg + 8, :], rop_t[pg:pg + 8, :], 0.75)
                    eng.scalar_tensor_tensor(
                        out=rw4[pg:pg + 8, :, :, 0],
                        in0=rop3[pg:pg + 8, :, 0:256],
                        scalar=0.25,
                        in1=t753[pg:pg + 8, :, 1:257],
                        op0=mybir.AluOpType.mult,
                        op1=mybir.AluOpType.add,
                    )
                    eng.scalar_tensor_tensor(
                        out=rw4[pg:pg + 8, :, :, 1],
                        in0=rop3[pg:pg + 8, :, 2:258],
                        scalar=0.25,
                        in1=t753[pg:pg + 8, :, 1:257],
                        op0=mybir.AluOpType.mult,
                        op1=mybir.AluOpType.add,
                    )

            # ================= PASS B: H-upsample + combine + store =================
            with (
                tc.tile_pool(name="passB", bufs=1) as bpool,
                tc.tile_pool(name="psB", bufs=2, space="PSUM") as psB,
            ):
                rwx = rw_t[:].rearrange("p (f x) -> p f x", x=W)
                for t in range(4):
                    fm_t = fmpool.tile([128, W * C], dt, tag="fm")
                    fm3 = fm_t[:].rearrange("p (x c) -> p x c", c=C)
                    nc.sync.dma_start(out=fm3, in_=fm_d[128 * t:128 * (t + 1)])

                    outqs = []
                    for q in range(4):
                        outq_t = bpool.tile([128, 128 * CH_OUT], dt, tag=f"out{q}")
                        outq3 = outq_t[:].rearrange("p (x ch) -> p x ch", ch=CH_OUT)
                        nc.scalar.activation(
                            out=outq3[:, :, 0:16],
                            in_=fm3[:, 128 * q:128 * (q + 1), :],
                            func=mybir.ActivationFunctionType.Copy,
                        )
                        outqs.append(outq3)

                    for b in range(2):
                        pg = 32 * b
                        lhsT = hup_t[pg:pg + 8, 128 * t:128 * (t + 1)]  # [8, 128]
                        for fq in range(4):  # f-quads
                            ps = psB.tile([128, 4 * W], dt, tag="up")
                            psf = ps[:].rearrange("p (f x) -> p f x", x=W)
                            for fi in range(4):
                                nc.tensor.matmul(
                                    psf[:, fi, :],
                                    lhsT,
                                    rwx[pg:pg + 8, fq * 4 + fi, :],
                                    start=True, stop=True,
                                )
                            psx = ps[:].rearrange("p (f x) -> p x f", x=W)
                            for q in range(4):
                                nc.vector.tensor_sub(
                                    outqs[q][:, :, 16 * (b + 1) + 4 * fq:
                                             16 * (b + 1) + 4 * fq + 4],
                                    fm3[:, 128 * q:128 * (q + 1), 4 * fq:4 * fq + 4],
                                    psx[:, 128 * q:128 * (q + 1), :],
                                )
                    for q in range(4):
                        nc.sync.dma_start(
                            out=out_d[128 * t:128 * (t + 1), 128 * q:128 * (q + 1), :],
                            in_=outqs[q],
                        )
    if compile:
        nc.compile()
    return nc


def _get_program():
    if "nc" not in _cache:
        _cache["nc"] = _build_program()
    return _cache["nc"]


def kernel(feature_map, kernel, bias):
    from concourse.bass_utils import run_bass_kernel_spmd

    feature_map = np.ascontiguousarray(feature_map, dtype=np.float32)
    kernel = np.ascontiguousarray(kernel, dtype=np.float32)
    bias = np.ascontiguousarray(bias, dtype=np.float32)
    B = feature_map.shape[0]
    assert B == 8

    poolw, hup, kt, bias2, _, _ = _host_consts(kernel, bias)
    nc = _get_program()
    in_maps = [
        {
            "feature_map": feature_map[b],
            "poolw": poolw,
            "hup": hup,
            "ktaps": kt,
            "bias2": bias2,
        }
        for b in range(B)
    ]
    res = run_bass_kernel_spmd(nc, in_maps, list(range(B)))
    out = np.stack([res.results[b]["out"] for b in range(B)])
    return out



# revision 2
# speedup vs baseline: 1.1788x; 1.1788x over previous
"""Trainium2 Bass kernel for nn_InpaintContextAttentionUnit.

Per-sample computation (B=8 samples -> 1 per NeuronCore):
  fm [512,512,16] -> avgpool(64x2) -> pooled [8,256,16]
  -> two masked 3x3 convs (middle row / middle col of kernel zeroed) + bias + relu
  -> bilinear upsample back to [512,512,16] (separable; half-pixel centers, edge clamp)
  -> out [512,512,48] = concat(fm, fm - row_up, fm - col_up)

Design (v2 — single fm read, contiguous DVE paths, DMA-overlap-first):
  - fm is loaded from HBM exactly once (4x 4 MiB fp32 HWDGE loads); ACT casts each
    tile into a persistent bf16 copy used by pooling AND by the pass-B subtract /
    passthrough (bf16 roundtrip error ~2^-9 rel, well under the 2e-2 gate)
  - pooling: PE matmul with a [128,2] block-mean matrix; rhs viewed (c, xp, par) so
    PSUM comes out (c, wp)-major -> the PSUM->SBUF copy is contiguous (the v1 copy
    was a transposed-AP CAST on 2 partitions costing 18us each); pooled rows DMA
    straight to the DRAM bounce buffer per tile
  - conv: per (branch, n-pair chunk): zero-init matmul + ~6 accumulating
    [16c,16f]x[16c,<=512] matmuls in PSUM; relu+bias on ACT; per-branch DRAM
    bounce so branch-0 W-upsample (DVE) overlaps branch-1 conv (PE)
  - W-upsample (x2): 2 scalar_tensor_tensor ops per branch computing
    pad[k]*1/3 + pad[k+1] (the 0.75 factor is folded into the host hup matrix,
    exact in bf16), written (x, f)-major into rw
  - H-upsample (x64): PE matmuls rw[8n, (x,f)] with host-built 0.75*HUp matrix;
    (x, f)-major PSUM makes the subtract reads contiguous
  - combine: DVE subtract (bf16 fm - psum) + ACT copy into interleaved
    [y, x, 48ch] staging tiles (bufs=4), contiguous 3 MiB DMAs out
All constant matrices are precomputed on host and passed as extra inputs.
"""

import numpy as np
import ml_dtypes

H, W, C, F = 512, 512, 16, 16
NPOOL = 8
WP = W // 2  # 256
CH_OUT = 3 * C  # 48

_cache = {}


def _host_consts(kernel, bias):
    """Build host-side constant matrices (bf16 for the PE-side constants)."""
    bf = ml_dtypes.bfloat16
    # pooling weights: [128, 2], 1/128 (exact in bf16) where row block matches
    poolw = np.zeros((128, 2), np.float32)
    poolw[:64, 0] = 1.0 / 128.0
    poolw[64:, 1] = 1.0 / 128.0
    # H-upsample matrix: hup[n, y] = weight of pooled row n for output row y,
    # scaled by 0.75 (the W-upsample major tap; k/64*0.75 = 3k/256 exact in bf16)
    hup = np.zeros((NPOOL, H), np.float32)
    scale = H // NPOOL
    for y in range(H):
        yf = (y + 0.5) / scale - 0.5
        i0 = int(np.floor(yf))
        w = yf - i0
        hup[min(max(i0, 0), NPOOL - 1), y] += 1.0 - w
        hup[min(max(i0 + 1, 0), NPOOL - 1), y] += w
    hup *= 0.75
    hup2 = np.zeros((40, H), np.float32)
    hup2[0:8] = hup
    hup2[32:40] = hup  # col-branch copy at base partition 32; rows 8-15 stay zero
    # conv taps: branch 0 (row conv): K[dn+1, dwp+1]; branch 1 (col): K[dwp+1, dn+1]
    taps0 = [(dn, dwp) for dn in (-1, 1) for dwp in (-1, 0, 1)]
    taps1 = [(dn, dwp) for dwp in (-1, 1) for dn in (-1, 0, 1)]
    kt = np.zeros((16, 13 * 16), np.float32)  # [c, tap*16+f]; slot 12 = zeros
    for i, (dn, dwp) in enumerate(taps0):
        kt[:, i * 16:(i + 1) * 16] = kernel[dn + 1, dwp + 1]
    for i, (dn, dwp) in enumerate(taps1):
        kt[:, (6 + i) * 16:(7 + i) * 16] = kernel[dwp + 1, dn + 1]
    bias2 = np.ascontiguousarray(bias.reshape(16, 1)).astype(np.float32)
    return (poolw.astype(bf), hup2.astype(bf), kt.astype(bf), bias2, taps0, taps1)


def _build_program(compile=True):
    import concourse.bass as bass
    import concourse.bacc as bacc
    import concourse.mybir as mybir
    import concourse.tile as tile

    dt = mybir.dt.float32
    db = mybir.dt.bfloat16
    nc = bacc.Bacc()

    fm_d = nc.declare_dram_parameter("feature_map", [H, W, C], dt, isOutput=False)
    poolw_d = nc.declare_dram_parameter("poolw", [128, 2], db, isOutput=False)
    hup_d = nc.declare_dram_parameter("hup", [40, H], db, isOutput=False)
    ktaps_d = nc.declare_dram_parameter("ktaps", [16, 208], db, isOutput=False)
    bias_d = nc.declare_dram_parameter("bias2", [16, 1], dt, isOutput=False)
    out_d = nc.declare_dram_parameter("out", [H, W, CH_OUT], dt, isOutput=True)

    taps0 = [(dn, dwp) for dn in (-1, 1) for dwp in (-1, 0, 1)]
    taps1 = [(dn, dwp) for dwp in (-1, 1) for dn in (-1, 0, 1)]
    taps_by_branch = [taps0, taps1]

    with tile.TileContext(nc) as tc:
        with (
            tc.tile_pool(name="consts", bufs=1) as cpool,
            tc.tile_pool(name="persist", bufs=1) as ppool,
        ):
            # ---- load constants ----
            poolw_t = cpool.tile([128, 2], db)
            nc.sync.dma_start(out=poolw_t[:], in_=poolw_d[:])
            hup_t = cpool.tile([40, H], db)
            nc.sync.dma_start(out=hup_t[:], in_=hup_d[:])
            ktaps_t = cpool.tile([16, 208], db)
            nc.sync.dma_start(out=ktaps_t[:], in_=ktaps_d[:])
            bias_t = cpool.tile([16, 1], dt)
            nc.sync.dma_start(out=bias_t[:], in_=bias_d[:])

            # persistent bf16 fm copy: [128, (4 t, 512 x, 16 c)]
            fmb_t = ppool.tile([128, 4 * W * C], db)
            # rw [40, (512 x, 16 f)] bf16: partitions 0-7 row-branch, 32-39 col
            rw_t = ppool.tile([40, W * 16], db)

            # ================= PASS A: pooling + conv + W-upsample =================
            with (
                tc.tile_pool(name="passA", bufs=1) as apool,
                tc.tile_pool(name="dram", bufs=1, space="DRAM") as dpool,
            ):
                # pooled DRAM bounce: [8 n, (16 c, 258 wp)] bf16 with zero wp-halo
                ncw_dram = dpool.tile([NPOOL, 16 * 258], db)
                nd3 = ncw_dram[:].rearrange("n (c w) -> n c w", w=258)
                zsrc = hup_d[8:16, 0:16]  # [8, 16] zeros
                nc.sync.dma_start(out=nd3[:, :, 0:1], in_=zsrc)
                nc.sync.dma_start(out=nd3[:, :, 257:258], in_=zsrc)

                # pooling rhs view of fmb: (t, c, xp, par)
                fmr = fmb_t[:].rearrange(
                    "p (t xp par c) -> p t c xp par", t=4, par=2, c=16)

                with tc.tile_pool(name="psA", bufs=1, space="PSUM") as psA:
                    for t in range(4):
                        fmf = apool.tile([128, W * C], dt, tag="fmf", bufs=2)
                        fmf3 = fmf[:].rearrange("p (x c) -> p x c", c=C)
                        nc.sync.dma_start(out=fmf3, in_=fm_d[128 * t:128 * (t + 1)])
                        # bf16 cast on ACT (persistent copy for pooling + pass B)
                        nc.scalar.activation(
                            out=fmb_t[:, t * W * C:(t + 1) * W * C],
                            in_=fmf[:],
                            func=mybir.ActivationFunctionType.Copy,
                        )
                        # H-pool (y->n) + W-pair add via PSUM accumulation;
                        # ps is (c, xp)-major; each j covers one c-pair = 1 bank
                        ps = psA.tile([2, 16 * WP], dt, tag="pool")
                        for j in range(8):
                            for par in range(2):
                                nc.tensor.matmul(
                                    ps[:, 512 * j:512 * (j + 1)], poolw_t[:],
                                    fmr[:, t, 2 * j:2 * j + 2, :, par],
                                    start=(par == 0), stop=(par == 1),
                                )
                        # contiguous f32->bf16 copy, then straight to DRAM bounce
                        stage = apool.tile([2, 16 * WP], db, tag="stage", bufs=2)
                        nc.vector.tensor_copy(stage[:], ps[:])
                        stage3 = stage[:].rearrange("p (c w) -> p c w", w=WP)
                        nc.sync.dma_start(
                            out=nd3[2 * t:2 * t + 2, :, 1:257], in_=stage3)

                # pooled_T [16 c, (8 n, 258 wp)] read-back (n-direction zero
                # padding handled by clipped matmul n-ranges)
                tpad_t = apool.tile([16, NPOOL * 258], db)
                tpad3 = tpad_t[:].rearrange("p (n w) -> p n w", w=258)
                ncwd3 = ncw_dram[:].rearrange("n (c w) -> c n w", w=258)
                nc.sync.dma_start(out=tpad3, in_=ncwd3)

                # ---- conv branches + W-upsample, pipelined per branch ----
                conv_t = apool.tile([16, 2 * NPOOL * WP], db)
                conv_dram = dpool.tile([16, 2 * NPOOL * WP], db)
                cd4 = conv_dram[:].rearrange("f (b n w) -> b n f w", b=2, n=NPOOL)
                rop_t = apool.tile([40, 16 * 258], db)
                rop3 = rop_t[:].rearrange("p (f w) -> p f w", w=258)
                rwv = rw_t[:].rearrange("p (xp par f) -> p f par xp", par=2, f=16)

                with tc.tile_pool(name="psConv", bufs=4, space="PSUM") as psC:
                    for b in range(2):
                        for ch in range(4):  # n-pair chunks: n in {2ch, 2ch+1}
                            n0 = 2 * ch
                            ps = psC.tile([16, 2 * WP], dt, tag="conv")
                            # zero-init whole chunk (ktaps slot 12 = zeros)
                            nc.tensor.matmul(
                                ps[:], ktaps_t[:, 192:208],
                                tpad3[:, n0:n0 + 2, 1:257],
                                start=True, stop=False, skip_group_check=True,
                            )
                            pieces = []
                            for i, (dn, dwp) in enumerate(taps_by_branch[b]):
                                nlo = max(n0, -dn)
                                nhi = min(n0 + 2, NPOOL - dn)
                                if nhi <= nlo:
                                    continue
                                pieces.append((b * 6 + i, dn, dwp, nlo, nhi))
                            for k, (sl, dn, dwp, nlo, nhi) in enumerate(pieces):
                                nc.tensor.matmul(
                                    ps[:, (nlo - n0) * WP:(nhi - n0) * WP],
                                    ktaps_t[:, sl * 16:(sl + 1) * 16],
                                    tpad3[:, nlo + dn:nhi + dn, 1 + dwp:257 + dwp],
                                    start=False, stop=(k == len(pieces) - 1),
                                    skip_group_check=True,
                                )
                            nc.scalar.activation(
                                out=conv_t[:, (b * NPOOL + n0) * WP:
                                           (b * NPOOL + n0 + 2) * WP],
                                in_=ps[:],
                                func=mybir.ActivationFunctionType.Relu,
                                bias=bias_t[:, 0:1],
                            )
                        # branch bounce: [16 f, (n, wp)] -> [(b,n) parts, (f, wp)]
                        nc.sync.dma_start(
                            out=conv_dram[:, b * NPOOL * WP:(b + 1) * NPOOL * WP],
                            in_=conv_t[:, b * NPOOL * WP:(b + 1) * NPOOL * WP])
                        pg = 32 * b  # partition base: row->0, col->32
                        nc.sync.dma_start(out=rop3[pg:pg + 8, :, 1:257], in_=cd4[b])
                        # edge replicate (W clamp)
                        nc.vector.tensor_copy(
                            rop3[pg:pg + 8, :, 0:1], rop3[pg:pg + 8, :, 1:2])
                        nc.vector.tensor_copy(
                            rop3[pg:pg + 8, :, 257:258], rop3[pg:pg + 8, :, 256:257])
                        # W-upsample into (x, f)-major rw; 0.75 folded into hup:
                        #   rw[2k]   = pad[k]/3   + pad[k+1]
                        #   rw[2k+1] = pad[k+2]/3 + pad[k+1]
                        third = 1.0 / 3.0
                        nc.vector.scalar_tensor_tensor(
                            out=rwv[pg:pg + 8, :, 0, :],
                            in0=rop3[pg:pg + 8, :, 0:256],
                            scalar=third,
                            in1=rop3[pg:pg + 8, :, 1:257],
                            op0=mybir.AluOpType.mult,
                            op1=mybir.AluOpType.add,
                        )
                        nc.vector.scalar_tensor_tensor(
                            out=rwv[pg:pg + 8, :, 1, :],
                            in0=rop3[pg:pg + 8, :, 2:258],
                            scalar=third,
                            in1=rop3[pg:pg + 8, :, 1:257],
                            op0=mybir.AluOpType.mult,
                            op1=mybir.AluOpType.add,
                        )

            # ================= PASS B: H-upsample + combine + store =================
            with (
                tc.tile_pool(name="passB", bufs=1) as bpool,
                tc.tile_pool(name="psB", bufs=2, space="PSUM") as psB,
            ):
                fmb4 = fmb_t[:].rearrange("p (t x c) -> p t x c", t=4, c=16)
                rwx = rw_t[:].rearrange("p (x f) -> p x f", f=16)
                for t in range(4):
                    for q in range(4):
                        outq = bpool.tile([128, 128 * CH_OUT], dt, tag="outq", bufs=4)
                        outq3 = outq[:].rearrange("p (x ch) -> p x ch", ch=CH_OUT)
                        fmq = fmb4[:, t, 128 * q:128 * (q + 1), :]
                        nc.scalar.activation(
                            out=outq3[:, :, 0:16], in_=fmq,
                            func=mybir.ActivationFunctionType.Copy,
                        )
                        for b in range(2):
                            pg = 32 * b
                            lhsT = hup_t[pg:pg + 8, 128 * t:128 * (t + 1)]  # [8,128]
                            ps = psB.tile([128, 128 * 16], dt, tag="up")
                            for i in range(4):  # 32-x chunks: 512 f32 = 1 bank
                                nc.tensor.matmul(
                                    ps[:, 512 * i:512 * (i + 1)],
                                    lhsT,
                                    rwx[pg:pg + 8,
                                        128 * q + 32 * i:128 * q + 32 * (i + 1), :],
                                    start=True, stop=True,
                                )
                            ps3 = ps[:].rearrange("p (x f) -> p x f", f=16)
                            nc.vector.tensor_sub(
                                outq3[:, :, 16 * (b + 1):16 * (b + 2)], fmq, ps3)
                        nc.sync.dma_start(
                            out=out_d[128 * t:128 * (t + 1),
                                      128 * q:128 * (q + 1), :],
                            in_=outq3,
                        )
    if compile:
        nc.compile()
    return nc


def _get_program():
    if "nc" not in _cache:
        _cache["nc"] = _build_program()
    return _cache["nc"]


def kernel(feature_map, kernel, bias):
    from concourse.bass_utils import run_bass_kernel_spmd

    feature_map = np.ascontiguousarray(feature_map, dtype=np.float32)
    kernel = np.ascontiguousarray(kernel, dtype=np.float32)
    bias = np.ascontiguousarray(bias, dtype=np.float32)
    B = feature_map.shape[0]
    assert B == 8

    poolw, hup, kt, bias2, _, _ = _host_consts(kernel, bias)
    nc = _get_program()
    in_maps = [
        {
            "feature_map": feature_map[b],
            "poolw": poolw,
            "hup": hup,
            "ktaps": kt,
            "bias2": bias2,
        }
        for b in range(B)
    ]
    res = run_bass_kernel_spmd(nc, in_maps, list(range(B)))
    out = np.stack([res.results[b]["out"] for b in range(B)])
    return out


# revision 7
# speedup vs baseline: 1.5343x; 1.3015x over previous
"""Trainium2 Bass kernel for nn_InpaintContextAttentionUnit.

Per-sample computation (B=8 samples -> 1 per NeuronCore):
  fm [512,512,16] -> avgpool(64x2) -> pooled [8,256,16]
  -> two masked 3x3 convs (middle row / middle col of kernel zeroed) + bias + relu
  -> bilinear upsample back to [512,512,16] (separable; half-pixel centers, edge clamp)
  -> out [512,512,48] = concat(fm, fm - row_up, fm - col_up)

Design (v2 — single fm read, contiguous DVE paths, DMA-overlap-first):
  - fm is loaded from HBM exactly once (4x 4 MiB fp32 HWDGE loads); ACT casts each
    tile into a persistent bf16 copy used by pooling AND by the pass-B subtract /
    passthrough (bf16 roundtrip error ~2^-9 rel, well under the 2e-2 gate)
  - pooling: PE matmul with a [128,2] block-mean matrix; rhs kept (xp, c)-inner-
    contiguous (strided rhs costs ~5 cyc/col on PE); contiguous CAST to bf16
    stage, SBUF-hop to an assembled ncw [8n, (xp c)]; the c<->w free transpose
    runs ONCE on 8 partitions (split DVE/ACT halves) instead of 4x on 2
    partitions (v1: 18us each)
  - conv: per (branch, n-pair chunk): zero-init matmul + ~6 accumulating
    [16c,16f]x[16c,<=512] matmuls in PSUM; relu+bias on ACT; per-branch DRAM
    bounce so branch-0 W-upsample (DVE) overlaps branch-1 conv (PE)
  - W-upsample (x2): 2 scalar_tensor_tensor ops per branch computing
    pad[k]*1/3 + pad[k+1] (the 0.75 factor is folded into the host hup matrix,
    exact in bf16), written (f, x)-major (x-inner iteration is the fast STT path)
  - H-upsample (x64): PE matmuls rw[8n, (f, x)] with host-built 0.75*HUp matrix
  - combine: DVE subtract (bf16 fm - psum) + ACT copy into interleaved
    [y, x, 48ch] staging tiles (bufs=4), contiguous 3 MiB DMAs out
All constant matrices are precomputed on host and passed as extra inputs.
"""

import numpy as np
import ml_dtypes

H, W, C, F = 512, 512, 16, 16
NPOOL = 8
WP = W // 2  # 256
CH_OUT = 3 * C  # 48

_cache = {}


def _host_consts(kernel, bias):
    """Build host-side constant matrices (bf16 for the PE-side constants)."""
    bf = ml_dtypes.bfloat16
    # pooling weights: [128, 2], 1/128 (exact in bf16) where row block matches
    poolw = np.zeros((128, 2), np.float32)
    poolw[:64, 0] = 1.0 / 128.0
    poolw[64:, 1] = 1.0 / 128.0
    # H-upsample matrix: hup[n, y] = weight of pooled row n for output row y,
    # scaled by 0.75 (the W-upsample major tap; k/64*0.75 = 3k/256 exact in bf16)
    hup = np.zeros((NPOOL, H), np.float32)
    scale = H // NPOOL
    for y in range(H):
        yf = (y + 0.5) / scale - 0.5
        i0 = int(np.floor(yf))
        w = yf - i0
        hup[min(max(i0, 0), NPOOL - 1), y] += 1.0 - w
        hup[min(max(i0 + 1, 0), NPOOL - 1), y] += w
    hup *= 0.75
    hup2 = np.zeros((40, H), np.float32)
    hup2[0:8] = hup
    hup2[32:40] = hup  # col-branch copy at base partition 32; rows 8-15 stay zero
    # conv taps: branch 0 (row conv): K[dn+1, dwp+1]; branch 1 (col): K[dwp+1, dn+1]
    taps0 = [(dn, dwp) for dn in (-1, 1) for dwp in (-1, 0, 1)]
    taps1 = [(dn, dwp) for dwp in (-1, 1) for dn in (-1, 0, 1)]
    kt = np.zeros((16, 13 * 16), np.float32)  # [c, tap*16+f]; slot 12 = zeros
    for i, (dn, dwp) in enumerate(taps0):
        kt[:, i * 16:(i + 1) * 16] = kernel[dn + 1, dwp + 1]
    for i, (dn, dwp) in enumerate(taps1):
        kt[:, (6 + i) * 16:(7 + i) * 16] = kernel[dwp + 1, dn + 1]
    bias2 = np.ascontiguousarray(bias.reshape(16, 1)).astype(np.float32)
    return (poolw.astype(bf), hup2.astype(bf), kt.astype(bf), bias2, taps0, taps1)


def _build_program(compile=True):
    import concourse.bass as bass
    import concourse.bacc as bacc
    import concourse.mybir as mybir
    import concourse.tile as tile

    dt = mybir.dt.float32
    db = mybir.dt.bfloat16
    nc = bacc.Bacc()

    fm_d = nc.declare_dram_parameter("feature_map", [H, W, C], dt, isOutput=False)
    poolw_d = nc.declare_dram_parameter("poolw", [128, 2], db, isOutput=False)
    hup_d = nc.declare_dram_parameter("hup", [40, H], db, isOutput=False)
    ktaps_d = nc.declare_dram_parameter("ktaps", [16, 208], db, isOutput=False)
    bias_d = nc.declare_dram_parameter("bias2", [16, 1], dt, isOutput=False)
    out_d = nc.declare_dram_parameter("out", [H, W, CH_OUT], dt, isOutput=True)

    taps0 = [(dn, dwp) for dn in (-1, 1) for dwp in (-1, 0, 1)]
    taps1 = [(dn, dwp) for dwp in (-1, 1) for dn in (-1, 0, 1)]
    taps_by_branch = [taps0, taps1]

    with tile.TileContext(nc) as tc:
        with (
            tc.tile_pool(name="consts", bufs=1) as cpool,
            tc.tile_pool(name="persist", bufs=1) as ppool,
        ):
            # ---- load constants ----
            poolw_t = cpool.tile([128, 2], db)
            nc.sync.dma_start(out=poolw_t[:], in_=poolw_d[:])
            hup_t = cpool.tile([40, H], db)
            nc.sync.dma_start(out=hup_t[:], in_=hup_d[:])
            ktaps_t = cpool.tile([16, 208], db)
            nc.sync.dma_start(out=ktaps_t[:], in_=ktaps_d[:])
            bias_t = cpool.tile([16, 1], dt)
            nc.sync.dma_start(out=bias_t[:], in_=bias_d[:])

            # persistent bf16 fm copy: [128, (4 t, 512 x, 16 c)]
            fmb_t = ppool.tile([128, 4 * W * C], db)
            # rw [40, (16 f, 512 x)] bf16: partitions 0-7 row-branch, 32-39 col
            rw_t = ppool.tile([40, 16 * W], db)

            # ================= PASS A: pooling + conv + W-upsample =================
            with (
                tc.tile_pool(name="passA", bufs=1) as apool,
                tc.tile_pool(name="dram", bufs=1, space="DRAM") as dpool,
            ):
                # pooled DRAM bounce: [8 n, (16 c, 258 wp)] bf16 with zero wp-halo
                ncw_dram = dpool.tile([NPOOL, 16 * 258], db)
                nd3 = ncw_dram[:].rearrange("n (c w) -> n c w", w=258)
                zsrc = hup_d[8:16, 0:16]  # [8, 16] zeros
                nc.sync.dma_start(out=nd3[:, :, 0:1], in_=zsrc)
                nc.sync.dma_start(out=nd3[:, :, 257:258], in_=zsrc)

                # pooling rhs view of fmb: (t, xp, par, c) — c-inner contiguous
                fmr = fmb_t[:].rearrange(
                    "p (t xp par c) -> p t xp par c", t=4, par=2, c=16)

                # pooled (xp, c)-major assembled across tiles: [8 n, (256 xp, 16 c)]
                ncw_t = apool.tile([NPOOL, WP * 16], db)

                with tc.tile_pool(name="psA", bufs=1, space="PSUM") as psA:
                    for t in range(4):
                        fmf = apool.tile([128, W * C], dt, tag="fmf", bufs=2)
                        fmf3 = fmf[:].rearrange("p (x c) -> p x c", c=C)
                        nc.sync.dma_start(out=fmf3, in_=fm_d[128 * t:128 * (t + 1)])
                        # bf16 cast on ACT (persistent copy for pooling + pass B)
                        nc.scalar.activation(
                            out=fmb_t[:, t * W * C:(t + 1) * W * C],
                            in_=fmf[:],
                            func=mybir.ActivationFunctionType.Copy,
                        )
                        # H-pool (y->n) + W-pair add via PSUM accumulation;
                        # ps is (xp, c)-major; each j = 32-xp block = 1 bank
                        ps = psA.tile([2, WP * 16], dt, tag="pool")
                        for j in range(8):
                            for par in range(2):
                                nc.tensor.matmul(
                                    ps[:, 512 * j:512 * (j + 1)], poolw_t[:],
                                    fmr[:, t, 32 * j:32 * (j + 1), par, :],
                                    start=(par == 0), stop=(par == 1),
                                )
                        # contiguous f32->bf16 copy + SBUF hop to ncw rows
                        stage = apool.tile([2, WP * 16], db, tag="stage", bufs=2)
                        nc.vector.tensor_copy(stage[:], ps[:])
                        nc.sync.dma_start(
                            out=ncw_t[2 * t:2 * t + 2, :], in_=stage[:])

                # free-dim transpose (xp, c) -> (c, w) once on 8 partitions,
                # split across DVE and ACT
                ncwT_t = apool.tile([NPOOL, 16 * WP], db)
                ncw_cx = ncw_t[:].rearrange("p (x c) -> p c x", c=16)
                ncwT3 = ncwT_t[:].rearrange("p (c x) -> p c x", x=WP)
                nc.vector.tensor_copy(ncwT3[:, 0:8, :], ncw_cx[:, 0:8, :])
                nc.scalar.activation(
                    out=ncwT3[:, 8:16, :], in_=ncw_cx[:, 8:16, :],
                    func=mybir.ActivationFunctionType.Copy,
                )
                nc.sync.dma_start(
                    out=nd3[:, :, 1:257],
                    in_=ncwT3)

                # pooled_T [16 c, (8 n, 258 wp)] read-back (n-direction zero
                # padding handled by clipped matmul n-ranges)
                tpad_t = apool.tile([16, NPOOL * 258], db)
                tpad3 = tpad_t[:].rearrange("p (n w) -> p n w", w=258)
                ncwd3 = ncw_dram[:].rearrange("n (c w) -> c n w", w=258)
                nc.sync.dma_start(out=tpad3, in_=ncwd3)

                # ---- conv branches + W-upsample, pipelined per branch ----
                conv_t = apool.tile([16, 2 * NPOOL * WP], db)
                conv_dram = dpool.tile([16, 2 * NPOOL * WP], db)
                cd4 = conv_dram[:].rearrange("f (b n w) -> b n f w", b=2, n=NPOOL)
                rop_t = apool.tile([40, 16 * 258], db)
                rop3 = rop_t[:].rearrange("p (f w) -> p f w", w=258)
                rwv = rw_t[:].rearrange("p (f xp par) -> p f par xp", par=2, xp=WP)

                with tc.tile_pool(name="psConv", bufs=4, space="PSUM") as psC:
                    for b in range(2):
                        for ch in range(4):  # n-pair chunks: n in {2ch, 2ch+1}
                            n0 = 2 * ch
                            ps = psC.tile([16, 2 * WP], dt, tag="conv")
                            # zero-init whole chunk (ktaps slot 12 = zeros)
                            nc.tensor.matmul(
                                ps[:], ktaps_t[:, 192:208],
                                tpad3[:, n0:n0 + 2, 1:257],
                                start=True, stop=False, skip_group_check=True,
                            )
                            pieces = []
                            for i, (dn, dwp) in enumerate(taps_by_branch[b]):
                                nlo = max(n0, -dn)
                                nhi = min(n0 + 2, NPOOL - dn)
                                if nhi <= nlo:
                                    continue
                                pieces.append((b * 6 + i, dn, dwp, nlo, nhi))
                            for k, (sl, dn, dwp, nlo, nhi) in enumerate(pieces):
                                nc.tensor.matmul(
                                    ps[:, (nlo - n0) * WP:(nhi - n0) * WP],
                                    ktaps_t[:, sl * 16:(sl + 1) * 16],
                                    tpad3[:, nlo + dn:nhi + dn, 1 + dwp:257 + dwp],
                                    start=False, stop=(k == len(pieces) - 1),
                                    skip_group_check=True,
                                )
                            nc.scalar.activation(
                                out=conv_t[:, (b * NPOOL + n0) * WP:
                                           (b * NPOOL + n0 + 2) * WP],
                                in_=ps[:],
                                func=mybir.ActivationFunctionType.Relu,
                                bias=bias_t[:, 0:1],
                            )
                        # branch bounce: [16 f, (n, wp)] -> [(b,n) parts, (f, wp)]
                        nc.sync.dma_start(
                            out=conv_dram[:, b * NPOOL * WP:(b + 1) * NPOOL * WP],
                            in_=conv_t[:, b * NPOOL * WP:(b + 1) * NPOOL * WP])
                        pg = 32 * b  # partition base: row->0, col->32
                        nc.sync.dma_start(out=rop3[pg:pg + 8, :, 1:257], in_=cd4[b])
                        # edge replicate (W clamp)
                        nc.vector.tensor_copy(
                            rop3[pg:pg + 8, :, 0:1], rop3[pg:pg + 8, :, 1:2])
                        nc.vector.tensor_copy(
                            rop3[pg:pg + 8, :, 257:258], rop3[pg:pg + 8, :, 256:257])
                        # W-upsample into (x, f)-major rw; 0.75 folded into hup:
                        #   rw[2k]   = pad[k]/3   + pad[k+1]
                        #   rw[2k+1] = pad[k+2]/3 + pad[k+1]
                        third = 1.0 / 3.0
                        nc.vector.scalar_tensor_tensor(
                            out=rwv[pg:pg + 8, :, 0, :],
                            in0=rop3[pg:pg + 8, :, 0:256],
                            scalar=third,
                            in1=rop3[pg:pg + 8, :, 1:257],
                            op0=mybir.AluOpType.mult,
                            op1=mybir.AluOpType.add,
                        )
                        nc.vector.scalar_tensor_tensor(
                            out=rwv[pg:pg + 8, :, 1, :],
                            in0=rop3[pg:pg + 8, :, 2:258],
                            scalar=third,
                            in1=rop3[pg:pg + 8, :, 1:257],
                            op0=mybir.AluOpType.mult,
                            op1=mybir.AluOpType.add,
                        )

            # ================= PASS B: H-upsample + combine + store =================
            with (
                tc.tile_pool(name="passB", bufs=1) as bpool,
                tc.tile_pool(name="psB", bufs=2, space="PSUM") as psB,
            ):
                fmb4 = fmb_t[:].rearrange("p (t x c) -> p t x c", t=4, c=16)
                rwx = rw_t[:].rearrange("p (f x) -> p f x", x=W)
                for t in range(4):
                    for q in range(4):
                        outq = bpool.tile([128, 128 * CH_OUT], dt, tag="outq", bufs=4)
                        outq3 = outq[:].rearrange("p (x ch) -> p x ch", ch=CH_OUT)
                        fmq = fmb4[:, t, 128 * q:128 * (q + 1), :]
                        nc.scalar.activation(
                            out=outq3[:, :, 0:16], in_=fmq,
                            func=mybir.ActivationFunctionType.Copy,
                        )
                        for b in range(2):
                            pg = 32 * b
                            lhsT = hup_t[pg:pg + 8, 128 * t:128 * (t + 1)]  # [8,128]
                            ps = psB.tile([128, 128 * 16], dt, tag="up")
                            # ps is (f, x-slice)-major: [128, (4f, 128x)] per bank
                            for i in range(4):  # 4-f chunks: 512 f32 = 1 bank
                                nc.tensor.matmul(
                                    ps[:, 512 * i:512 * (i + 1)],
                                    lhsT,
                                    rwx[pg:pg + 8, 4 * i:4 * (i + 1),
                                        128 * q:128 * (q + 1)],
                                    start=True, stop=True,
                                )
                            psx = ps[:].rearrange("p (f x) -> p x f", x=128)
                            nc.vector.tensor_sub(
                                outq3[:, :, 16 * (b + 1):16 * (b + 2)], fmq, psx)
                        nc.sync.dma_start(
                            out=out_d[128 * t:128 * (t + 1),
                                      128 * q:128 * (q + 1), :],
                            in_=outq3,
                        )
    if compile:
        nc.compile()
    return nc


def _get_program():
    if "nc" not in _cache:
        _cache["nc"] = _build_program()
    return _cache["nc"]


def kernel(feature_map, kernel, bias):
    from concourse.bass_utils import run_bass_kernel_spmd

    feature_map = np.ascontiguousarray(feature_map, dtype=np.float32)
    kernel = np.ascontiguousarray(kernel, dtype=np.float32)
    bias = np.ascontiguousarray(bias, dtype=np.float32)
    B = feature_map.shape[0]
    assert B == 8

    poolw, hup, kt, bias2, _, _ = _host_consts(kernel, bias)
    nc = _get_program()
    in_maps = [
        {
            "feature_map": feature_map[b],
            "poolw": poolw,
            "hup": hup,
            "ktaps": kt,
            "bias2": bias2,
        }
        for b in range(B)
    ]
    res = run_bass_kernel_spmd(nc, in_maps, list(range(B)))
    out = np.stack([res.results[b]["out"] for b in range(B)])
    return out
